# revision 19
# baseline (speedup 1.0000x reference)
"""MinibatchDiscrimination Trainium2 kernel (8-core SPMD, full I/O).

Math (reference):
  act = einsum('bd,kdm->bkm', x, W)        # (512, 64, 16)
  l1[i,j,k] = sum_m |act[i,k,m] - act[j,k,m]|
  feats[i,k] = sum_j exp(-l1[i,j,k]) + b[k]
  out = concat([x, feats], axis=1)         # (512, 320)

Sharding: data-parallel over batch rows. Core c owns rows [64c, 64c+64);
it computes the full transposed activation actT (km=1024 partitions-tiled,
j=512 free) on-device from replicated x^T / W2, plus its own rows' columns
actIT from a per-core x_rows^T input, then its 64x512 block of pairwise L1.

Per-core pipeline (layout: partitions = k*16+m in 8 tiles of 128, free = j).
Rows are processed in pairs sharing one [128, 512] PSUM (parity 0 -> l1T in
partitions 0:64, parity 1 -> 64:128) so exp uses all 128 ACT lanes:
  DVE:  tiles 0..4: adif = actT_t - actIT_t[:,i] (tensor_scalar) + one
        batched sign-clear AND over the uint16 view (= abs, 4x tier)
  ACT:  tiles 5..7: adif = Abs(actT_t + (-actIT_t[:,i])) fused via the
        per-partition bias operand; then exp(-l1T) whose accum_out gives
        the j-sum directly -> featsP column (one exp per row pair)
  PE:   actT/actIT = W2^T-tile @ x (bf16); m-group-sum via block-mask
        matmuls accumulating l1T in PSUM; final 128x128 transpose
Engine balance measured on HW: ACT ~89%, DVE ~84%, PE ~60% busy.
"""

import sys

sys.path.insert(0, "/opt/trn_rl_repo")

import numpy as np

import concourse.bass as bass
import concourse.bacc as bacc
import concourse.tile as tile
from concourse import mybir
from concourse import bass_utils

B, D, K, M = 512, 256, 64, 16
KM = K * M          # 1024
NT = KM // 128      # 8 km-tiles
NCORES = 8
RPC = B // NCORES   # 64 rows per core

FP32 = mybir.dt.float32
BF16 = mybir.dt.bfloat16


def build_bass():
    nc = bacc.Bacc(None, target_bir_lowering=False, debug=False)

    xT = nc.declare_dram_parameter("xT", [D, B], FP32, isOutput=False)
    w2 = nc.declare_dram_parameter("w2", [D, KM], FP32, isOutput=False)
    xiT = nc.declare_dram_parameter("xiT", [D, RPC], FP32, isOutput=False)
    xi = nc.declare_dram_parameter("xi", [RPC, D], FP32, isOutput=False)
    brep = nc.declare_dram_parameter("brep", [RPC // 2, 2 * K], FP32, isOutput=False)
    gmask = nc.declare_dram_parameter("gmask", [128, NT, K], FP32, isOutput=False)
    ident = nc.declare_dram_parameter("ident", [128, 128], FP32, isOutput=False)
    out = nc.declare_dram_parameter("out", [RPC, D + K], FP32, isOutput=True)

    with tile.TileContext(nc) as tc:
        with (
            tc.tile_pool(name="consts", bufs=1) as consts,
            tc.tile_pool(name="work", bufs=3) as work,
            tc.tile_pool(name="small", bufs=2) as small,
            tc.tile_pool(name="psum_a", bufs=2, space="PSUM") as psum_a,
            tc.tile_pool(name="psum_l", bufs=2, space="PSUM") as psum_l,
        ):
            # ---- load + convert inputs ----
            xT_f = consts.tile([128, 2, B], FP32, tag="xT_f")
            nc.sync.dma_start(out=xT_f, in_=xT[:].rearrange("(h p) b -> p h b", p=128))
            w2_f = consts.tile([128, 2, KM], FP32, tag="w2_f")
            nc.sync.dma_start(out=w2_f, in_=w2[:].rearrange("(h p) n -> p h n", p=128))
            xiT_f = consts.tile([128, 2, RPC], FP32, tag="xiT_f")
            nc.sync.dma_start(out=xiT_f, in_=xiT[:].rearrange("(h p) b -> p h b", p=128))
            xi_f = consts.tile([RPC, D], FP32, tag="xi_f")
            nc.sync.dma_start(out=xi_f, in_=xi[:])
            brep_f = consts.tile([RPC // 2, 2 * K], FP32, tag="brep_f")
            nc.sync.dma_start(out=brep_f, in_=brep[:])
            gm_f = consts.tile([128, NT, K], FP32, tag="gm_f")
            nc.sync.dma_start(out=gm_f, in_=gmask[:])
            id_f = consts.tile([128, 128], FP32, tag="id_f")
            nc.sync.dma_start(out=id_f, in_=ident[:])

            xT_b = consts.tile([128, 2, B], BF16, tag="xT_b")
            nc.vector.tensor_copy(xT_b, xT_f)
            w2_b = consts.tile([128, 2, KM], BF16, tag="w2_b")
            nc.vector.tensor_copy(w2_b, w2_f)
            xiT_b = consts.tile([128, 2, RPC], BF16, tag="xiT_b")
            nc.vector.tensor_copy(xiT_b, xiT_f)
            gm_b = consts.tile([128, NT, K], BF16, tag="gm_b")
            nc.vector.tensor_copy(gm_b, gm_f)

            # ---- actT_t [128km, 512j] & actIT_t [128km, 64i] bf16, t=0..7 ----
            actT = []
            actIT = []
            negIT = []
            for t in range(NT):
                pa = psum_a.tile([128, B], FP32, tag="pa")
                for dh in range(2):
                    nc.tensor.matmul(
                        pa,
                        w2_b[:, dh, t * 128:(t + 1) * 128],
                        xT_b[:, dh, :],
                        start=(dh == 0),
                        stop=(dh == 1),
                    )
                aT = consts.tile([128, B], BF16, tag=f"actT{t}")
                nc.scalar.copy(aT, pa)
                actT.append(aT)

                pi = psum_a.tile([128, RPC], FP32, tag="pi")
                for dh in range(2):
                    nc.tensor.matmul(
                        pi,
                        w2_b[:, dh, t * 128:(t + 1) * 128],
                        xiT_b[:, dh, :],
                        start=(dh == 0),
                        stop=(dh == 1),
                    )
                aI = consts.tile([128, RPC], BF16, tag=f"actIT{t}")
                nc.scalar.copy(aI, pi)
                # fp32 copy of the bf16-rounded values: scalar1 must be fp32,
                # and matching actT's bf16 rounding keeps the i==j diagonal
                # exactly zero (exp(0)=1 dominates feats).
                aIf = consts.tile([128, RPC], FP32, tag=f"actITf{t}")
                nc.vector.tensor_copy(aIf, aI)
                actIT.append(aIf)
                nIf = consts.tile([128, RPC], FP32, tag=f"negIT{t}")
                nc.vector.tensor_scalar(
                    out=nIf, in0=aIf, scalar1=-1.0, scalar2=None,
                    op0=mybir.AluOpType.mult,
                )
                negIT.append(nIf)

            # ---- main pairwise loop ----
            # Two batch rows (a pair) share one [128, 512] PSUM: row parity 0
            # accumulates l1T into partitions 0:64, parity 1 into 64:128, so
            # the exp runs once per pair on all 128 lanes. Per row, DVE does
            # |actT - col| for ND tiles (tensor_scalar subtract + one batched
            # sign-clear AND); ACT does the rest via fused Abs(actT - col).
            ND = 5   # DVE tiles 0..4 (subtract + u16 sign-clear); ACT 5..7
            featsP = consts.tile([128, RPC // 2], FP32, tag="featsP")
            for pr in range(RPC // 2):
                pl = psum_l.tile([128, B], FP32, tag="pl")
                adifs = []
                for e in range(2):
                    il = 2 * pr + e
                    adifD = work.tile([128, ND, B], BF16, tag=f"adifD{e}")
                    adifA = work.tile([128, NT - ND, B], BF16, tag=f"adifA{e}")
                    for t in range(ND):
                        nc.vector.tensor_scalar(
                            out=adifD[:, t, :],
                            in0=actT[t],
                            scalar1=actIT[t][:, il:il + 1],
                            scalar2=None,
                            op0=mybir.AluOpType.subtract,
                        )
                    au = adifD.bitcast(mybir.dt.uint16)
                    nc.vector.tensor_scalar(
                        out=au, in0=au, scalar1=0x7FFF, scalar2=None,
                        op0=mybir.AluOpType.bitwise_and,
                    )
                    for t in range(ND, NT):
                        nc.scalar.activation(
                            out=adifA[:, t - ND, :],
                            in_=actT[t],
                            func=mybir.ActivationFunctionType.Abs,
                            bias=negIT[t][:, il:il + 1],
                            scale=1.0,
                        )
                    adifs.append((adifD, adifA))
                for e in range(2):
                    adifD, adifA = adifs[e]
                    for t in range(NT):
                        src = adifD[:, t, :] if t < ND else adifA[:, t - ND, :]
                        nc.tensor.matmul(
                            pl[e * K:(e + 1) * K, :], gm_b[:, t, :], src,
                            start=(t == 0), stop=(t == NT - 1),
                        )
                scr = small.tile([128, B], BF16, tag="scr")
                nc.scalar.activation(
                    out=scr,
                    in_=pl,
                    func=mybir.ActivationFunctionType.Exp,
                    scale=-1.0,
                    accum_out=featsP[:, pr:pr + 1],
                )

            # ---- feats rows: transpose featsP, add b, write out ----
            ptr = psum_a.tile([RPC // 2, 128], FP32, tag="ptr")
            nc.tensor.transpose(ptr, featsP, id_f)
            outf = consts.tile([RPC // 2, 2 * K], FP32, tag="outf")
            nc.vector.tensor_tensor(
                out=outf, in0=ptr, in1=brep_f, op=mybir.AluOpType.add
            )
            nc.sync.dma_start(
                out=out[:, D:D + K].rearrange("(c e) k -> c e k", e=2),
                in_=outf.rearrange("c (e k) -> c e k", e=2),
            )
            nc.sync.dma_start(out=out[:, 0:D], in_=xi_f)

    nc.compile()
    return nc


_NC_CACHE = None


def _get_nc():
    global _NC_CACHE
    if _NC_CACHE is None:
        _NC_CACHE = build_bass()
    return _NC_CACHE


def make_in_maps(x, W, b):
    x = np.asarray(x, dtype=np.float32)
    W = np.asarray(W, dtype=np.float32)
    b = np.asarray(b, dtype=np.float32)
    xT = np.ascontiguousarray(x.T)                       # (256, 512)
    w2 = np.ascontiguousarray(W.transpose(1, 0, 2).reshape(D, KM))
    brep = np.ascontiguousarray(np.broadcast_to(np.tile(b, 2)[None, :], (RPC // 2, 2 * K)))
    # gmask[p, t, k] = 1 iff k == 8*t + p//16  (m-group membership)
    p = np.arange(128)[:, None, None]
    t = np.arange(NT)[None, :, None]
    k = np.arange(K)[None, None, :]
    gmask = (k == NT * t + p // M).astype(np.float32)
    ident = np.eye(128, dtype=np.float32)
    in_maps = []
    for c in range(NCORES):
        rows = slice(c * RPC, (c + 1) * RPC)
        in_maps.append({
            "xT": xT,
            "w2": w2,
            "xiT": np.ascontiguousarray(x[rows].T),      # (256, 64)
            "xi": np.ascontiguousarray(x[rows]),         # (64, 256)
            "brep": brep,
            "gmask": gmask,
            "ident": ident,
        })
    return in_maps


def kernel(x, W, b, _trace=False, _tmpdir=None):
    nc = _get_nc()
    in_maps = make_in_maps(x, W, b)
    res = bass_utils.run_bass_kernel_spmd(
        nc, in_maps, core_ids=list(range(NCORES)),
        trace=_trace, tmpdir=_tmpdir,
    )
    out = np.concatenate([res.results[c]["out"] for c in range(NCORES)], axis=0)
    if _trace:
        return out, res
    return out


# revision 20
# speedup vs baseline: 1.0423x; 1.0423x over previous
"""MinibatchDiscrimination Trainium2 kernel (8-core SPMD, full I/O).

Math (reference):
  act = einsum('bd,kdm->bkm', x, W)        # (512, 64, 16)
  l1[i,j,k] = sum_m |act[i,k,m] - act[j,k,m]|
  feats[i,k] = sum_j exp(-l1[i,j,k]) + b[k]
  out = concat([x, feats], axis=1)         # (512, 320)

Sharding: data-parallel over batch rows. Core c owns rows [64c, 64c+64);
it computes the full transposed activation actT (km=1024 partitions-tiled,
j=512 free) on-device from replicated x^T / W2, plus its own rows' columns
actIT from a per-core x_rows^T input, then its 64x512 block of pairwise L1.

Per-core pipeline (layout: partitions = k*16+m in 8 tiles of 128, free = j).
Rows are processed in pairs sharing one [128, 512] PSUM (parity 0 -> l1T in
partitions 0:64, parity 1 -> 64:128) so exp uses all 128 ACT lanes:
  DVE:  tiles 0..4: adif = actT_t - actIT_t[:,i] (tensor_scalar) + one
        batched sign-clear AND over the uint16 view (= abs, 4x tier)
  ACT:  tiles 5..7: adif = Abs(actT_t + (-actIT_t[:,i])) fused via the
        per-partition bias operand; then exp(-l1T) whose accum_out gives
        the j-sum directly -> featsP column (one exp per row pair)
  PE:   actT/actIT = W2^T-tile @ x (bf16); m-group-sum via block-mask
        matmuls accumulating l1T in PSUM; final 128x128 transpose
Engine balance measured on HW: ACT ~89%, DVE ~84%, PE ~60% busy.
"""

import sys

sys.path.insert(0, "/opt/trn_rl_repo")

import numpy as np

import concourse.bass as bass
import concourse.bacc as bacc
import concourse.tile as tile
from concourse import mybir
from concourse import bass_utils

B, D, K, M = 512, 256, 64, 16
KM = K * M          # 1024
NT = KM // 128      # 8 km-tiles
NCORES = 8
RPC = B // NCORES   # 64 rows per core

FP32 = mybir.dt.float32
BF16 = mybir.dt.bfloat16


def build_bass():
    nc = bacc.Bacc(None, target_bir_lowering=False, debug=False)

    xT = nc.declare_dram_parameter("xT", [D, B], FP32, isOutput=False)
    w2 = nc.declare_dram_parameter("w2", [D, KM], FP32, isOutput=False)
    xiT = nc.declare_dram_parameter("xiT", [D, RPC], FP32, isOutput=False)
    xi = nc.declare_dram_parameter("xi", [RPC, D], FP32, isOutput=False)
    brep = nc.declare_dram_parameter("brep", [RPC // 2, 2 * K], FP32, isOutput=False)
    gmask = nc.declare_dram_parameter("gmask", [128, NT, K], FP32, isOutput=False)
    ident = nc.declare_dram_parameter("ident", [128, 128], FP32, isOutput=False)
    out = nc.declare_dram_parameter("out", [RPC, D + K], FP32, isOutput=True)

    with tile.TileContext(nc) as tc:
        with (
            tc.tile_pool(name="consts", bufs=1) as consts,
            tc.tile_pool(name="work", bufs=3) as work,
            tc.tile_pool(name="small", bufs=2) as small,
            tc.tile_pool(name="psum_a", bufs=2, space="PSUM") as psum_a,
            tc.tile_pool(name="psum_l", bufs=2, space="PSUM") as psum_l,
        ):
            # ---- load + convert inputs ----
            xT_f = consts.tile([128, 2, B], FP32, tag="xT_f")
            nc.sync.dma_start(out=xT_f, in_=xT[:].rearrange("(h p) b -> p h b", p=128))
            w2_f = consts.tile([128, 2, KM], FP32, tag="w2_f")
            nc.sync.dma_start(out=w2_f, in_=w2[:].rearrange("(h p) n -> p h n", p=128))
            xiT_f = consts.tile([128, 2, RPC], FP32, tag="xiT_f")
            nc.sync.dma_start(out=xiT_f, in_=xiT[:].rearrange("(h p) b -> p h b", p=128))
            xi_f = consts.tile([RPC, D], FP32, tag="xi_f")
            nc.sync.dma_start(out=xi_f, in_=xi[:])
            brep_f = consts.tile([RPC // 2, 2 * K], FP32, tag="brep_f")
            nc.sync.dma_start(out=brep_f, in_=brep[:])
            gm_f = consts.tile([128, NT, K], FP32, tag="gm_f")
            nc.sync.dma_start(out=gm_f, in_=gmask[:])
            id_f = consts.tile([128, 128], FP32, tag="id_f")
            nc.sync.dma_start(out=id_f, in_=ident[:])

            xT_b = consts.tile([128, 2, B], BF16, tag="xT_b")
            nc.vector.tensor_copy(xT_b, xT_f)
            w2_b = consts.tile([128, 2, KM], BF16, tag="w2_b")
            nc.vector.tensor_copy(w2_b, w2_f)
            xiT_b = consts.tile([128, 2, RPC], BF16, tag="xiT_b")
            nc.vector.tensor_copy(xiT_b, xiT_f)
            gm_b = consts.tile([128, NT, K], BF16, tag="gm_b")
            nc.vector.tensor_copy(gm_b, gm_f)

            # ---- actT_t [128km, 512j] & actIT_t [128km, 64i] bf16, t=0..7 ----
            actT = []
            actIT = []
            negIT = []
            for t in range(NT):
                pa = psum_a.tile([128, B], FP32, tag="pa")
                for dh in range(2):
                    nc.tensor.matmul(
                        pa,
                        w2_b[:, dh, t * 128:(t + 1) * 128],
                        xT_b[:, dh, :],
                        start=(dh == 0),
                        stop=(dh == 1),
                    )
                aT = consts.tile([128, B], BF16, tag=f"actT{t}")
                nc.scalar.copy(aT, pa)
                actT.append(aT)

                pi = psum_a.tile([128, RPC], FP32, tag="pi")
                for dh in range(2):
                    nc.tensor.matmul(
                        pi,
                        w2_b[:, dh, t * 128:(t + 1) * 128],
                        xiT_b[:, dh, :],
                        start=(dh == 0),
                        stop=(dh == 1),
                    )
                aI = consts.tile([128, RPC], BF16, tag=f"actIT{t}")
                nc.scalar.copy(aI, pi)
                # fp32 copy of the bf16-rounded values: scalar1 must be fp32,
                # and matching actT's bf16 rounding keeps the i==j diagonal
                # exactly zero (exp(0)=1 dominates feats).
                aIf = consts.tile([128, RPC], FP32, tag=f"actITf{t}")
                nc.vector.tensor_copy(aIf, aI)
                actIT.append(aIf)
                nIf = consts.tile([128, RPC], FP32, tag=f"negIT{t}")
                nc.vector.tensor_scalar(
                    out=nIf, in0=aIf, scalar1=-1.0, scalar2=None,
                    op0=mybir.AluOpType.mult,
                )
                negIT.append(nIf)

            # ---- main pairwise loop ----
            # Two batch rows (a pair) share one [128, 512] PSUM: row parity 0
            # accumulates l1T into partitions 0:64, parity 1 into 64:128, so
            # the exp runs once per pair on all 128 lanes. Per row, DVE does
            # |actT - col| for ND tiles (tensor_scalar subtract + one batched
            # sign-clear AND); ACT does the rest via fused Abs(actT - col).
            ND = 5   # DVE tiles 0..4 (subtract + u16 sign-clear); ACT 5..7
            featsP = consts.tile([128, RPC // 2], FP32, tag="featsP")
            for pr in range(RPC // 2):
                pl = psum_l.tile([128, B], FP32, tag="pl")
                adifs = []
                for e in range(2):
                    il = 2 * pr + e
                    # fractional rebalance: every 3rd pair, row e=0 shifts
                    # one abs tile from ACT (its busiest) to DVE
                    nd = ND + 1 if (e == 0 and pr % 3 == 0) else ND
                    adifD = work.tile([128, ND + 1, B], BF16, tag=f"adifD{e}")
                    adifA = work.tile([128, NT - ND, B], BF16, tag=f"adifA{e}")
                    for t in range(nd):
                        nc.vector.tensor_scalar(
                            out=adifD[:, t, :],
                            in0=actT[t],
                            scalar1=actIT[t][:, il:il + 1],
                            scalar2=None,
                            op0=mybir.AluOpType.subtract,
                        )
                    au = adifD[:, 0:nd, :].bitcast(mybir.dt.uint16)
                    nc.vector.tensor_scalar(
                        out=au, in0=au, scalar1=0x7FFF, scalar2=None,
                        op0=mybir.AluOpType.bitwise_and,
                    )
                    for t in range(nd, NT):
                        nc.scalar.activation(
                            out=adifA[:, t - ND, :],
                            in_=actT[t],
                            func=mybir.ActivationFunctionType.Abs,
                            bias=negIT[t][:, il:il + 1],
                            scale=1.0,
                        )
                    adifs.append((adifD, adifA, nd))
                for e in range(2):
                    adifD, adifA, nd = adifs[e]
                    for t in range(NT):
                        src = adifD[:, t, :] if t < nd else adifA[:, t - ND, :]
                        nc.tensor.matmul(
                            pl[e * K:(e + 1) * K, :], gm_b[:, t, :], src,
                            start=(t == 0), stop=(t == NT - 1),
                        )
                scr = small.tile([128, B], BF16, tag="scr")
                nc.scalar.activation(
                    out=scr,
                    in_=pl,
                    func=mybir.ActivationFunctionType.Exp,
                    scale=-1.0,
                    accum_out=featsP[:, pr:pr + 1],
                )

            # ---- feats rows: transpose featsP, add b, write out ----
            ptr = psum_a.tile([RPC // 2, 128], FP32, tag="ptr")
            nc.tensor.transpose(ptr, featsP, id_f)
            outf = consts.tile([RPC // 2, 2 * K], FP32, tag="outf")
            nc.vector.tensor_tensor(
                out=outf, in0=ptr, in1=brep_f, op=mybir.AluOpType.add
            )
            nc.sync.dma_start(
                out=out[:, D:D + K].rearrange("(c e) k -> c e k", e=2),
                in_=outf.rearrange("c (e k) -> c e k", e=2),
            )
            nc.sync.dma_start(out=out[:, 0:D], in_=xi_f)

    nc.compile()
    return nc


_NC_CACHE = None


def _get_nc():
    global _NC_CACHE
    if _NC_CACHE is None:
        _NC_CACHE = build_bass()
    return _NC_CACHE


def make_in_maps(x, W, b):
    x = np.asarray(x, dtype=np.float32)
    W = np.asarray(W, dtype=np.float32)
    b = np.asarray(b, dtype=np.float32)
    xT = np.ascontiguousarray(x.T)                       # (256, 512)
    w2 = np.ascontiguousarray(W.transpose(1, 0, 2).reshape(D, KM))
    brep = np.ascontiguousarray(np.broadcast_to(np.tile(b, 2)[None, :], (RPC // 2, 2 * K)))
    # gmask[p, t, k] = 1 iff k == 8*t + p//16  (m-group membership)
    p = np.arange(128)[:, None, None]
    t = np.arange(NT)[None, :, None]
    k = np.arange(K)[None, None, :]
    gmask = (k == NT * t + p // M).astype(np.float32)
    ident = np.eye(128, dtype=np.float32)
    in_maps = []
    for c in range(NCORES):
        rows = slice(c * RPC, (c + 1) * RPC)
        in_maps.append({
            "xT": xT,
            "w2": w2,
            "xiT": np.ascontiguousarray(x[rows].T),      # (256, 64)
            "xi": np.ascontiguousarray(x[rows]),         # (64, 256)
            "brep": brep,
            "gmask": gmask,
            "ident": ident,
        })
    return in_maps


def kernel(x, W, b, _trace=False, _tmpdir=None):
    nc = _get_nc()
    in_maps = make_in_maps(x, W, b)
    res = bass_utils.run_bass_kernel_spmd(
        nc, in_maps, core_ids=list(range(NCORES)),
        trace=_trace, tmpdir=_tmpdir,
    )
    out = np.concatenate([res.results[c]["out"] for c in range(NCORES)], axis=0)
    if _trace:
        return out, res
    return out
